# revision 42
# baseline (speedup 1.0000x reference)
"""Trainium2 Bass kernel for nn_Attention (dense_transformer, ridge regime).

Computation per batch b:
    scores[s]  = <lstm_output[b,s,:], hidden[b,:]>          # [S]
    w          = softmax(scores)                            # [S]
    attn[h]    = sum_s w[s] * lstm_output[b,s,h]            # [H]
    out[b]     = [hidden[b], attn] @ W_combine.T + b_combine

Sharding: data-parallel over batch B=64 across 8 cores (8 batches/core).

v3: fp16 on-chip pipeline (host casts lstm/hidden/W to fp16; f32 score
accumulation and PSUM, f32 output -> rel err ~1.3e-3 vs the 2e-2 gate):
  - DMA traffic ~36MB/core; L(b) arrives in 1MB quarters and the L(b+2)
    load is released per-quarter by einsum2(b) progress, so the DMA
    never waits on a whole-batch dependency.
  - scores: DVE multiplies TWO tiles per tensor_tensor (fp16 2x mode,
    [128,2,1024] pairs), and the 16 per-tile reductions are spread over
    THREE engines: ACT accum (tiles 0-6,15), GPSIMD reduce (8-14), DVE
    reduce (7).  Keeps every engine at ~10-12.5us/batch.
  - hidden replication hidR2[128,b,2,H] comes from broadcast DMAs on the
    ACT HWDGE queue (no compute); the x2 free-dim copy feeds the 2-wide
    multiplies.
  - einsum2 and the final projection are fp16 matmuls; W^T streams in
    eight 0.5MB chunks interleaved between batch loads.
  - drains only where a same-engine RAW is 1 op away (exp after negMcp).
"""

import numpy as np

import concourse.bass as bass
from concourse import bass_isa, library_config, mybir
from concourse.bass_utils import run_bass_kernel_spmd

F32 = mybir.dt.float32
F16 = mybir.dt.float16

B, S, H = 64, 2048, 1024
NCORES = 8
BPC = B // NCORES          # batches per core
T = S // 128               # s-tiles per batch
NQ = 4                     # quarters per batch load
TPQ = T // NQ              # tiles per quarter
NPAIR = T // 2             # 2-tile mult pairs
NCH = (2 * H) // 128       # 16 chunks of the combined dim
HCH = H // 128             # 8 chunks of one H

DVE_RED = (5, 6)           # pairs {10,11},{12,13} reduced by DVE (axis=X)
ACT_TILES = tuple(t for t in range(16) if t // 2 not in DVE_RED)  # 12 tiles
NSLOT = 3                  # prod pair-slot ring

_cached_nc = None
last_results = None


def _build_program():
    nc = bass.Bass()

    lstm_d = nc.declare_dram_parameter("lstm_output", [BPC, S, H], F16, isOutput=False)
    hid_d = nc.declare_dram_parameter("hidden", [BPC, H], F16, isOutput=False)
    wt_d = nc.declare_dram_parameter("w_t", [2 * H, H], F16, isOutput=False)
    b_d = nc.declare_dram_parameter("b_combine", [H], F32, isOutput=False)
    out_d = nc.declare_dram_parameter("out", [BPC, H], F32, isOutput=True)

    # ---- SBUF ----
    L = [nc.alloc_sbuf_tensor(f"L{i}", [128, T, H], F16) for i in range(2)]  # 2x4MB
    WT = nc.alloc_sbuf_tensor("WT", [128, NCH, H], F16)                      # 4MB
    hid_t = nc.alloc_sbuf_tensor("hid", [BPC, H], F16)
    hid = hid_t.ap()
    bias_t = nc.alloc_sbuf_tensor("bias", [BPC, H], F32)
    bias = bias_t.ap()
    out_t = nc.alloc_sbuf_tensor("out_sb", [BPC, H], F32)
    out_sb = out_t.ap()
    hidR2 = nc.alloc_sbuf_tensor("hidR2", [128, BPC, 2, H], F16)             # 4MB
    prod = [nc.alloc_sbuf_tensor(f"prod{i}", [128, 2, H], F16) for i in range(NSLOT)]
    dmy = nc.alloc_sbuf_tensor("dmy", [128, T], F32)
    CT = nc.alloc_sbuf_tensor("CT", [128, NCH, BPC], F16)                    # combined^T
    scores = [nc.alloc_sbuf_tensor(f"scores{i}", [128, T], F32) for i in range(2)]
    wexp = [nc.alloc_sbuf_tensor(f"wexp{i}", [128, T], F16) for i in range(2)]
    zp = [nc.alloc_sbuf_tensor(f"zp{i}", [128, 1], F32) for i in range(2)]
    mp = nc.alloc_sbuf_tensor("mp", [128, 1], F32)
    negM1_t = nc.alloc_sbuf_tensor("negM1s", [1, 2], F32)
    negM1 = [negM1_t.ap()[0:1, i:i + 1] for i in range(2)]
    negM = [nc.alloc_sbuf_tensor(f"negM{i}", [128, 1], F32) for i in range(2)]
    rZ_t = nc.alloc_sbuf_tensor("rZs", [1, 2], F32)
    rZ = [rZ_t.ap()[0:1, i:i + 1] for i in range(2)]
    ones128 = nc.alloc_sbuf_tensor("ones128", [128, 1], F32)
    attn2 = nc.alloc_sbuf_tensor("attn2", [1, 2 * H], F32)
    attn_sb = [attn2.ap()[0:1, i * H:(i + 1) * H] for i in range(2)]
    ones_colf = nc.alloc_sbuf_tensor("ones_colf", [1, 128], F32)   # bcast lhsT / attnT ident
    identf = nc.alloc_sbuf_tensor("identf", [128, 128], F32)       # mp transpose
    identh = nc.alloc_sbuf_tensor("identh", [BPC, BPC], F16)       # hidT transpose

    # ---- PSUM: one bank per concurrent PE write target ----
    acc_lo = nc.alloc_psum_tensor("acc_lo", [BPC, 512], F32)
    acc_hi = nc.alloc_psum_tensor("acc_hi", [BPC, 512], F32)
    ct8_t = nc.alloc_psum_tensor("ct8", [128, HCH, BPC], F16)
    ctc_t = nc.alloc_psum_tensor("ctc", [128, 512], F32)
    mpT_t = nc.alloc_psum_tensor("mpT", [1, 128], F32)
    negM_t = nc.alloc_psum_tensor("negMbc", [128, 1], F32)
    Zps_t = nc.alloc_psum_tensor("Zps", [1, 1], F32)
    mpT = mpT_t.ap()
    negM_bc = negM_t.ap()
    Zps = Zps_t.ap()
    ctcols8 = ct8_t.ap()
    ctcols = ctc_t.ap()[:, 0:HCH]

    # ---------------- two-pass emission ----------------
    ev = {}
    sems = {}
    counts = {}

    RAW_SEMS = ("pe", "dve", "act", "gps", "hid", "bias", "l0", "l1",
                "wt", "outd", "hidr")

    class Prog:
        def __init__(self, name):
            self.name = name
            self.emit = False
            self.eng = None
            self.hwm = {}
            self.auto_drain = name in ("dve", "act", "gps")
            self.first_op = True

        def begin(self, eng=None, emit=False):
            self.emit = emit
            self.eng = eng
            self.hwm = {}
            self.first_op = True

        def wait(self, key):
            """key: event tuple, or (sem_name, value) pair."""
            if len(key) == 2 and isinstance(key[1], int) and key[0] in RAW_SEMS:
                sname, val = key
            else:
                if self.emit and key not in ev:
                    raise KeyError(f"wait on unknown event {key}")
                sname, val = ev.get(key, (None, 0))
            if val <= 0 or sname is None:
                return
            if self.hwm.get(sname, -1) >= val:
                return
            self.hwm[sname] = val
            if self.emit:
                self.eng.wait_ge(sems[sname], val)

        def op(self, fn, inc=1, sem=None, drain=None):
            sname = sem or self.name
            counts[sname] = counts.get(sname, 0) + inc
            if self.emit:
                do_drain = self.auto_drain if drain is None else drain
                if do_drain and not self.first_op:
                    self.eng.drain()
                inst = fn()
                inst.then_inc(sems[sname], inc)
            self.first_op = False

        def mark(self, *key, sem=None):
            sname = sem or self.name
            ev[(self.name,) + tuple(key)] = (sname, counts.get(sname, 0))

    DMA, PE, DVE, ACT, GPS = Prog("dma"), Prog("pe"), Prog("dve"), Prog("act"), Prog("gps")

    bias_src = b_d[:]
    bias_bcast = bass.AP(
        tensor=bias_src.tensor,
        offset=bias_src.offset,
        ap=[[0, BPC]] + list(bias_src.ap),
    )

    def hidr2_bcast(b0, b1):
        """DRAM source AP replicating hidden[b0:b1] to [128, b1-b0, H]."""
        src = hid_d[b0:b1]
        return bass.AP(
            tensor=src.tensor,
            offset=src.offset,
            ap=[[0, 128]] + [list(src.ap[0])] + [list(src.ap[1])],
        )

    def prog_gps():
        g = GPS.eng if GPS.emit else None
        GPS.op(lambda: g.memset(ones_colf.ap(), 1.0))
        GPS.op(lambda: g.memset(ones128.ap(), 1.0))
        GPS.op(lambda: g.memset(identf.ap(), 0.0))
        GPS.op(lambda: g.affine_select(
            out=identf.ap(), in_=identf.ap(),
            compare_op=mybir.AluOpType.not_equal, fill=1.0, base=0,
            pattern=[[-1, 128]], channel_multiplier=1))
        GPS.op(lambda: g.memset(identh.ap(), 0.0), drain=True)
        GPS.op(lambda: g.affine_select(
            out=identh.ap(), in_=identh.ap(),
            compare_op=mybir.AluOpType.not_equal, fill=1.0, base=0,
            pattern=[[-1, BPC]], channel_multiplier=1), drain=True)
        GPS.mark("setup")

    def prog_dma():
        d = DMA.eng if DMA.emit else None
        DMA.op(lambda: d.dma_start(out=hid, in_=hid_d[:]), inc=16, sem="hid")
        DMA.mark("hid", sem="hid")
        DMA.op(lambda: d.dma_start(out=bias, in_=bias_bcast), inc=16, sem="bias")
        DMA.mark("bias", sem="bias")
        for b in range(BPC):
            src = lstm_d[b].rearrange("(t p) h -> p t h", p=128)
            for q in range(NQ):
                if b >= 2:
                    DMA.wait(("pe", "e2q", b - 2, q))
                DMA.op(lambda src=src, b=b, q=q: d.dma_start(
                    out=L[b % 2].ap()[:, TPQ * q:TPQ * (q + 1), :],
                    in_=src[:, TPQ * q:TPQ * (q + 1), :]),
                    inc=16, sem=f"l{b % 2}")
                DMA.mark("L", b, q, sem=f"l{b % 2}")
            # interleave one 0.5MB W^T chunk per batch slot
            wt_src = wt_d[:].rearrange("(c p) n -> p c n", p=128)
            DMA.op(lambda b=b: d.dma_start(
                out=WT.ap()[:, 2 * b:2 * (b + 1), :],
                in_=wt_src[:, 2 * b:2 * (b + 1), :]), inc=16, sem="wt")
            DMA.mark("wt", b, sem="wt")
        DMA.wait(("dve", "bias_hi"))
        DMA.op(lambda: d.dma_start(out=out_d[:], in_=out_sb), inc=16, sem="outd")
        DMA.wait(("outd", counts.get("outd", 0)))

    def prog_pe():
        p = PE.eng if PE.emit else None
        PE.wait(("gps", "setup"))
        PE.wait(("hid", 16))
        # hidden^T -> CT chunks 0..7 staging (psum)
        for c in range(HCH):
            PE.op(lambda c=c: p.transpose(
                ctcols8[:, c, :], hid[0:BPC, c * 128:(c + 1) * 128],
                identh.ap()))
        PE.mark("hidT")
        # batch 0 max-chain head
        PE.wait(("dve", "rmax", 0))
        PE.op(lambda: p.transpose(mpT, mp.ap(), identf.ap()))
        PE.mark("transp", 0)
        for b in range(BPC):
            PE.wait(("dve", "rmax2", b))
            PE.op(lambda b=b: p.matmul(
                negM_bc, lhsT=ones_colf.ap(), rhs=negM1[b % 2],
                start=True, stop=True))
            PE.mark("bcast", b)
            if b >= 1:
                PE.wait(("dve", "recip", b - 1))   # Zps consumed
            PE.wait(("act", "exp", b))
            PE.op(lambda b=b: p.matmul(
                Zps, lhsT=zp[b % 2].ap(), rhs=ones128.ap(),
                start=True, stop=True))
            PE.mark("z", b)
            # einsum2: attn_unnorm = sum_s w[s] * L[s, :]
            if b >= 1:
                PE.wait(("dve", "cplo", b - 1))    # acc_lo/hi consumed
                PE.wait(("act", "cphi", b - 1))
            for t in range(T):
                PE.op(lambda b=b, t=t: p.matmul(
                    acc_lo.ap()[0:1, :],
                    lhsT=wexp[b % 2].ap()[:, t:t + 1],
                    rhs=L[b % 2].ap()[:, t, 0:512],
                    start=(t == 0), stop=(t == T - 1)))
                PE.op(lambda b=b, t=t: p.matmul(
                    acc_hi.ap()[0:1, :],
                    lhsT=wexp[b % 2].ap()[:, t:t + 1],
                    rhs=L[b % 2].ap()[:, t, 512:1024],
                    start=(t == 0), stop=(t == T - 1)))
                if t % TPQ == TPQ - 1:
                    PE.mark("e2q", b, t // TPQ)
            PE.mark("e2", b)
            if b + 1 < BPC:
                PE.wait(("dve", "rmax", b + 1))
                PE.op(lambda: p.transpose(mpT, mp.ap(), identf.ap()))
                PE.mark("transp", b + 1)
            # attn row -> CT columns (chunk transposes via K=1 matmuls)
            PE.wait(("dve", "cplo", b))
            PE.wait(("act", "cphi", b))
            if b >= 1:
                PE.wait(("act", "ctcp", b - 1))
            for c in range(HCH):
                PE.op(lambda b=b, c=c: p.transpose(
                    ctcols[:, c:c + 1],
                    attn_sb[b % 2][0:1, c * 128:(c + 1) * 128],
                    ones_colf.ap()[0:1, 0:1]))
            PE.mark("attnT", b)
        # final projection
        PE.wait(("act", "ctcp", BPC - 1))
        PE.wait(("dve", "cth"))
        PE.wait(("wt", counts.get("wt", 0)))
        for c in range(NCH):
            PE.op(lambda c=c: p.matmul(
                acc_lo.ap()[0:BPC, :],
                lhsT=CT.ap()[:, c, :],
                rhs=WT.ap()[:, c, 0:512],
                start=(c == 0), stop=(c == NCH - 1)))
            PE.op(lambda c=c: p.matmul(
                acc_hi.ap()[0:BPC, :],
                lhsT=CT.ap()[:, c, :],
                rhs=WT.ap()[:, c, 512:1024],
                start=(c == 0), stop=(c == NCH - 1)))
        PE.mark("final")

    def prog_dve():
        v = DVE.eng if DVE.emit else None
        # CT hidden columns: psum staging -> CT
        DVE.wait(("pe", "hidT"))
        DVE.op(lambda: v.tensor_copy(CT.ap()[:, 0:HCH, :], ctcols8))
        DVE.mark("cth")
        for b in range(BPC):
            DVE.wait(("hidr", 32 if b < 2 else 64))
            if b >= 2:
                DVE.wait(("act", "exp", b - 2))    # scores slot reuse
            # mult stream: 16 one-wide mults (fp16 2x) into pair-slot
            # halves prod[(t//2)%3][:, t%2, :]; pair reduces for {10,11}
            # and {12,13} interleave with >=2 intervening ops (no drain)
            for t in range(T):
                p = t // 2
                DVE.wait(("dma", "L", b, t // TPQ))
                if t % 2 == 0 and p >= NSLOT and (p - NSLOT) not in DVE_RED:
                    DVE.wait(("act", "acc", b, 2 * (p - NSLOT) + 1))
                DVE.op(lambda b=b, t=t, p=p: v.tensor_mul(
                    prod[p % NSLOT].ap()[:, t % 2, :],
                    L[b % 2].ap()[:, t, :],
                    hidR2.ap()[:, b, 0, :]), drain=False)
                DVE.mark("mult", b, t)
                if t == 13:
                    DVE.op(lambda b=b: v.reduce_sum(
                        scores[b % 2].ap()[:, 10:12],
                        prod[5 % NSLOT].ap(),
                        axis=mybir.AxisListType.X), drain=False)
                if t == 2 and b >= 1:
                    DVE.wait(("pe", "z", b - 1))
                    DVE.op(lambda b=b: v.reciprocal(rZ[(b - 1) % 2], Zps),
                           drain=False)
                    DVE.mark("recip", b - 1)
                if t == 7 and b >= 1:
                    DVE.wait(("pe", "e2", b - 1))
                    DVE.op(lambda b=b: v.tensor_scalar_mul(
                        attn_sb[(b - 1) % 2][0:1, 0:512], acc_lo.ap()[0:1, :],
                        rZ[(b - 1) % 2]), drain=False)
                    DVE.mark("cplo", b - 1)
            DVE.op(lambda b=b: v.reduce_sum(
                scores[b % 2].ap()[:, 12:14],
                prod[6 % NSLOT].ap(),
                axis=mybir.AxisListType.X), drain=False)
            if b >= 1:
                DVE.wait(("pe", "transp", b - 1))   # mp slot reuse
            DVE.wait(("act", "acc", b, 15))
            DVE.op(lambda b=b: v.reduce_max(
                mp.ap(), scores[b % 2].ap(), axis=mybir.AxisListType.X),
                drain=True)
            DVE.mark("rmax", b)
            DVE.wait(("pe", "transp", b))
            DVE.op(lambda b=b: v.reduce_max(
                negM1[b % 2], mpT, axis=mybir.AxisListType.X, negate=True),
                drain=False)
            DVE.mark("rmax2", b)
        DVE.wait(("pe", "z", BPC - 1))
        DVE.op(lambda: v.reciprocal(rZ[(BPC - 1) % 2], Zps), drain=False)
        DVE.mark("recip", BPC - 1)
        DVE.wait(("pe", "e2", BPC - 1))
        DVE.op(lambda: v.tensor_scalar_mul(
            attn_sb[(BPC - 1) % 2][0:1, 0:512], acc_lo.ap()[0:1, :],
            rZ[(BPC - 1) % 2]), drain=True)
        DVE.mark("cplo", BPC - 1)
        # final bias adds
        DVE.wait(("pe", "final"))
        DVE.wait(("bias", 16))
        DVE.op(lambda: v.tensor_add(
            out_sb[:, 0:512], acc_lo.ap()[0:BPC, :], bias[:, 0:512]),
            drain=False)
        DVE.mark("bias_lo")
        DVE.op(lambda: v.tensor_add(
            out_sb[:, 512:1024], acc_hi.ap()[0:BPC, :], bias[:, 512:1024]),
            drain=False)
        DVE.mark("bias_hi")

    def prog_act():
        a = ACT.eng if ACT.emit else None
        Copy = mybir.ActivationFunctionType.Copy
        Exp = mybir.ActivationFunctionType.Exp
        # hidden replication via broadcast DMAs on the ACT HWDGE queue
        # (each batch range twice: the two halves of the x2 free-dim copy)
        for b0, b1 in ((0, 2), (2, BPC)):
            for h in range(2):
                ACT.op(lambda b0=b0, b1=b1, h=h: a.dma_start(
                    out=hidR2.ap()[:, b0:b1, h, :],
                    in_=hidr2_bcast(b0, b1)),
                    inc=16, sem="hidr", drain=False)
        NHEAD = 3   # accums of batch b+1 interleaved before exp(b)
        def accum(b, t, drain=False):
            ACT.wait(("dve", "mult", b, t))
            ACT.op(lambda b=b, t=t: a.activation(
                out=dmy.ap()[:, t:t + 1].broadcast_to((128, H)),
                in_=prod[(t // 2) % NSLOT].ap()[:, t % 2, :],
                func=Copy, accum_out=scores[b % 2].ap()[:, t:t + 1]),
                drain=drain)
            ACT.mark("acc", b, t)
        for t in ACT_TILES[:NHEAD]:
            accum(0, t, drain=(t == ACT_TILES[0]))
        for b in range(BPC):
            for t in ACT_TILES[NHEAD:]:
                accum(b, t)
            if b >= 2:
                ACT.wait(("pe", "attnT", b - 2))
                ACT.op(lambda b=b: a.activation(
                    out=CT.ap()[:, HCH:NCH, b - 2], in_=ctcols, func=Copy),
                    drain=False)
                ACT.mark("ctcp", b - 2)
            if b >= 1:
                ACT.wait(("pe", "e2", b - 1))
                ACT.wait(("dve", "recip", b - 1))
                ACT.op(lambda b=b: a.activation(
                    out=attn_sb[(b - 1) % 2][0:1, 512:1024], in_=acc_hi.ap()[0:1, :],
                    func=Copy, scale=rZ[(b - 1) % 2]), drain=False)
                ACT.mark("cphi", b - 1)
            # hide the max-chain hops behind the next batch's head accums
            if b + 1 < BPC:
                for t in ACT_TILES[:NHEAD]:
                    accum(b + 1, t)
            ACT.wait(("pe", "bcast", b))
            ACT.op(lambda b=b: a.activation(
                out=negM[b % 2].ap(), in_=negM_bc, func=Copy), drain=False)
            if b >= 2:
                ACT.wait(("pe", "e2", b - 2))    # wexp/zp slot reuse
            ACT.op(lambda b=b: a.activation(
                out=wexp[b % 2].ap(), in_=scores[b % 2].ap(), func=Exp,
                bias=negM[b % 2].ap(), scale=1.0, accum_out=zp[b % 2].ap()),
                drain=True)
            ACT.mark("exp", b)
        for b in (BPC - 1,):
            ACT.wait(("pe", "e2", b))
            ACT.wait(("dve", "recip", b))
            ACT.op(lambda b=b: a.activation(
                out=attn_sb[b % 2][0:1, 512:1024], in_=acc_hi.ap()[0:1, :],
                func=Copy, scale=rZ[b % 2]), drain=False)
            ACT.mark("cphi", b)
        for b in (BPC - 2, BPC - 1):
            ACT.wait(("pe", "attnT", b))
            ACT.op(lambda b=b: a.activation(
                out=CT.ap()[:, HCH:NCH, b], in_=ctcols, func=Copy),
                drain=False)
            ACT.mark("ctcp", b)

    progs = [
        (GPS, prog_gps), (DMA, prog_dma), (PE, prog_pe),
        (DVE, prog_dve), (ACT, prog_act),
    ]

    # pass 1: count
    for pr, fn in progs:
        pr.begin(emit=False)
        fn()

    # pass 2: emit
    counts.clear()
    sem_names = ["pe", "dve", "act", "gps", "hid", "bias", "l0", "l1", "wt",
                 "outd", "hidr"]
    with nc.Block() as block:
        for sn in sem_names:
            sems[sn] = nc.alloc_semaphore(name=f"{sn}_sem")

        @block.gpsimd
        def _(eng):
            GPS.begin(eng=eng, emit=True)
            prog_gps()

        @block.sync
        def _(eng):
            DMA.begin(eng=eng, emit=True)
            prog_dma()

        @block.tensor
        def _(eng):
            PE.begin(eng=eng, emit=True)
            prog_pe()

        @block.vector
        def _(eng):
            DVE.begin(eng=eng, emit=True)
            prog_dve()

        @block.scalar
        def _(eng):
            ACT.begin(eng=eng, emit=True)
            prog_act()

    return nc


def kernel(lstm_output, hidden, W_combine, b_combine):
    global _cached_nc, last_results
    lstm_output = np.asarray(lstm_output, dtype=np.float32)
    hidden = np.asarray(hidden, dtype=np.float32)
    W_combine = np.asarray(W_combine, dtype=np.float32)
    b_combine = np.asarray(b_combine, dtype=np.float32)

    if _cached_nc is None:
        _cached_nc = _build_program()
    nc = _cached_nc

    lstm_h = lstm_output.astype(np.float16)
    hid_h = hidden.astype(np.float16)
    wt_host = np.ascontiguousarray(W_combine.T.astype(np.float16))
    in_maps = []
    for i in range(NCORES):
        sl = slice(i * BPC, (i + 1) * BPC)
        in_maps.append({
            "lstm_output": np.ascontiguousarray(lstm_h[sl]),
            "hidden": np.ascontiguousarray(hid_h[sl]),
            "w_t": wt_host,
            "b_combine": b_combine,
        })
    res = run_bass_kernel_spmd(nc, in_maps, core_ids=list(range(NCORES)))
    last_results = res
    return np.concatenate([res.results[i]["out"] for i in range(NCORES)], axis=0)
